# revision 11
# baseline (speedup 1.0000x reference)
"""Trainium2 Bass kernel for nn_ConvSkip (GNN message passing layer).

Computes, for the full graph:
    h    = data @ W_lin                 (b_lin cancels in the laplacian)
    lap  = h - (1/deg) * sum_{j in N(i)} h_j
    out  = relu(lap + merge @ W_tr + b_tr)

Sharding: nodes are padded to 50176 and split into 8 contiguous shards of
6272 (49 node-tiles of 128). Every core receives the SAME pre-transposed
bf16 feature table dataT [128, 50176] and redundantly computes the full
transformed table h (bf16) into its own DRAM, written in a host-permuted
row order that makes every table write a contiguous full-bandwidth DMA.
Each core then processes its own shard: per supertile it issues one
indirect-DMA gather per (node-tile, neighbor-slot) — the HW consumes one
offset per partition — reduces the 16 neighbor rows, and accumulates
bias + merge @ W_tr + data_own @ W_lin (the laplacian self term) in PSUM.

Per-core differences live entirely in the inputs (idx / mergeT /
dataT_own / out slices), so the compiled program is identical on all
cores and no node rotation is needed.
"""

import numpy as np

P = 128
N_NODES = 50000
DEG = 16
D_IN = 128
D_OUT = 64
N_CORES = 8
PAD_N = 50176  # 392 tiles of 128
SHARD = PAD_N // N_CORES  # 6272 = 49 tiles

# phase-1 supertile: T1 node-tiles of 128 rows per step (4 PSUM groups of 7)
T1 = 28
N_ST1 = PAD_N // (T1 * P)  # 14
# phase-2 supertile: T2 node-tiles of 128 rows per step
T2 = 7
N_ST2 = SHARD // (T2 * P)  # 7


def build_nc(repeat=1):
    import concourse.bass as bass
    import concourse.tile as tile
    from concourse import bacc, mybir

    f32 = mybir.dt.float32
    bf16 = mybir.dt.bfloat16
    i32 = mybir.dt.int32

    nc = bacc.Bacc("TRN2", target_bir_lowering=False)

    dataT = nc.declare_dram_parameter("dataT", [P, PAD_N], bf16, isOutput=False)
    dataT_own = nc.declare_dram_parameter("dataT_own", [P, SHARD], bf16, isOutput=False)
    mergeT = nc.declare_dram_parameter("mergeT", [P, SHARD], bf16, isOutput=False)
    idx_r = nc.declare_dram_parameter("idx_r", [SHARD, DEG], i32, isOutput=False)
    w_lin = nc.declare_dram_parameter("w_lin", [D_IN, D_OUT], bf16, isOutput=False)
    w_tr = nc.declare_dram_parameter("w_tr", [D_IN, D_OUT], bf16, isOutput=False)
    btr_t = nc.declare_dram_parameter("btr_t", [1, T2 * D_OUT], bf16, isOutput=False)
    out_r = nc.declare_dram_parameter("out_r", [SHARD, D_OUT], bf16, isOutput=True)

    with tile.TileContext(nc) as tc:
        with (
            tc.tile_pool(name="const", bufs=1) as cpool,
            tc.tile_pool(name="sbuf", bufs=2) as pool,
            tc.tile_pool(name="gath", bufs=3) as gpool,
            tc.tile_pool(name="ld", bufs=3) as ldpool,
            tc.tile_pool(name="idxp", bufs=7) as idxpool,
            tc.tile_pool(name="mp", bufs=4) as mpool,
            tc.tile_pool(name="psum", bufs=2, space="PSUM") as psum,
            tc.tile_pool(name="psum2", bufs=4, space="PSUM") as psum2,
            tc.tile_pool(name="dram", bufs=1, space="DRAM") as dpool,
        ):
            # ---- constants ----
            w_lin_sb = cpool.tile([P, D_OUT], bf16)
            nc.sync.dma_start(out=w_lin_sb[:], in_=w_lin[:, :])
            w_tr_sb = cpool.tile([P, D_OUT], bf16)
            nc.sync.dma_start(out=w_tr_sb[:], in_=w_tr[:, :])
            ones1 = cpool.tile([1, P], bf16)
            nc.vector.memset(ones1[:], 1.0)
            btr_sb = cpool.tile([1, T2 * D_OUT], bf16)
            nc.sync.dma_start(out=btr_sb[:], in_=btr_t[:, :])

            # full transformed-feature table in this core's DRAM (bf16)
            h_table = dpool.tile([PAD_N, D_OUT], bf16)

            def body():
                # ---- phase 1: h table over all nodes ----
                for st in range(N_ST1):
                    r0 = st * T1 * P
                    xT = ldpool.tile([P, T1 * P], bf16, tag="ld1")
                    nc.sync.dma_start(out=xT[:], in_=dataT[:, r0 : r0 + T1 * P])
                    stage = pool.tile([P, T1, D_OUT], bf16, tag="stage")
                    for g in range(4):
                        h_ps = psum.tile([P, T2 * D_OUT], f32, tag="mm")
                        for t in range(T2):
                            j = g * T2 + t
                            nc.tensor.matmul(
                                out=h_ps[:, t * D_OUT : (t + 1) * D_OUT],
                                lhsT=xT[:, j * P : (j + 1) * P],
                                rhs=w_lin_sb[:],
                                start=True,
                                stop=True,
                            )
                        if (st * 4 + g) % 2 == 0:
                            nc.vector.tensor_copy(
                                out=stage[:, g * T2 : (g + 1) * T2, :],
                                in_=h_ps[:].rearrange("p (t f) -> p t f", t=T2),
                            )
                        else:
                            nc.scalar.copy(
                                out=stage[:, g * T2 : (g + 1) * T2, :],
                                in_=h_ps[:].rearrange("p (t f) -> p t f", t=T2),
                            )
                    # permuted table layout: node n = r0 + t*128 + p is stored
                    # at DRAM row r0 + p*T1 + t, making each partition's write
                    # a single contiguous 3584B chunk (full DMA bandwidth).
                    # The host permutes gather indices to match.
                    nc.scalar.dma_start(
                        out=h_table[r0 : r0 + T1 * P, :].rearrange(
                            "(p t) f -> p t f", p=P
                        ),
                        in_=stage[:],
                    )

                # ---- phase 2: gather + laplacian + skip for own shard ----
                for su in range(N_ST2):
                    r0 = su * T2 * P
                    idx_sb = idxpool.tile([P, T2, DEG], i32, tag="idx")
                    nc.sync.dma_start(
                        out=idx_sb[:],
                        in_=idx_r[r0 : r0 + T2 * P, :].rearrange("(t p) d -> p t d", p=P),
                    )
                    m_sb = mpool.tile([P, T2 * P], bf16, tag="ldm")
                    nc.sync.dma_start(out=m_sb[:], in_=mergeT[:, r0 : r0 + T2 * P])
                    o_sb = mpool.tile([P, T2 * P], bf16, tag="ldo")
                    nc.sync.dma_start(out=o_sb[:], in_=dataT_own[:, r0 : r0 + T2 * P])

                    # gather: HW indirect DMA consumes ONE offset per partition
                    # (streams contiguously per partition), so issue one gather
                    # per (node-tile, neighbor-slot): [128,1] idx -> [128,64].
                    gath = gpool.tile([P, T2 * DEG * D_OUT], bf16, tag="gath")
                    for t in range(T2):
                        for dd in range(DEG):
                            k = t * DEG + dd
                            nc.gpsimd.indirect_dma_start(
                                out=gath[:, k * D_OUT : (k + 1) * D_OUT],
                                out_offset=None,
                                in_=h_table[:, :],
                                in_offset=bass.IndirectOffsetOnAxis(
                                    ap=idx_sb[:, t, dd : dd + 1], axis=0
                                ),
                            )

                    # skip branch + laplacian self term, accumulated in PSUM:
                    # sk = b_tr + merge @ W_tr + data_own @ W_lin
                    sk_ps = psum2.tile([P, T2 * D_OUT], f32, tag="sk")
                    nc.tensor.matmul(
                        out=sk_ps[:],
                        lhsT=ones1[:],
                        rhs=btr_sb[:],
                        start=True,
                        stop=False,
                        skip_group_check=True,
                    )
                    for t in range(T2):
                        nc.tensor.matmul(
                            out=sk_ps[:, t * D_OUT : (t + 1) * D_OUT],
                            lhsT=m_sb[:, t * P : (t + 1) * P],
                            rhs=w_tr_sb[:],
                            start=False,
                            stop=False,
                            skip_group_check=True,
                        )
                        nc.tensor.matmul(
                            out=sk_ps[:, t * D_OUT : (t + 1) * D_OUT],
                            lhsT=o_sb[:, t * P : (t + 1) * P],
                            rhs=w_lin_sb[:],
                            start=False,
                            stop=(t == T2 - 1),
                            skip_group_check=True,
                        )

                    nsum = pool.tile([P, T2, D_OUT], f32, tag="nsum")
                    nc.vector.reduce_sum(
                        out=nsum[:],
                        in_=gath[:].rearrange("p (t d f) -> p t f d", t=T2, d=DEG),
                        axis=mybir.AxisListType.X,
                    )
                    osum = pool.tile([P, T2, D_OUT], f32, tag="osum")
                    nc.vector.scalar_tensor_tensor(
                        out=osum[:],
                        in0=nsum[:],
                        scalar=-1.0 / DEG,
                        in1=sk_ps[:].rearrange("p (t f) -> p t f", t=T2),
                        op0=mybir.AluOpType.mult,
                        op1=mybir.AluOpType.add,
                    )
                    orelu = pool.tile([P, T2, D_OUT], bf16, tag="orelu")
                    nc.scalar.activation(
                        out=orelu[:],
                        in_=osum[:],
                        func=mybir.ActivationFunctionType.Relu,
                    )
                    # permuted output rows (node su*896 + t*128 + p -> row
                    # su*896 + p*T2 + t); host un-permutes. 896B contiguous
                    # per partition avoids the small-descriptor DMA penalty.
                    nc.scalar.dma_start(
                        out=out_r[r0 : r0 + T2 * P, :].rearrange("(p t) f -> p t f", p=P),
                        in_=orelu[:],
                    )

            for _rep in range(repeat):
                body()

    nc.finalize()
    return nc


def _make_in_maps(data, merge, structure, W_lin, W_tr, b_tr):
    import ml_dtypes

    bf16 = ml_dtypes.bfloat16

    data_f = np.ascontiguousarray(data, dtype=np.float32)
    dataT = np.zeros((P, PAD_N), dtype=bf16)
    dataT[:, :N_NODES] = np.ascontiguousarray(data_f.T).astype(bf16)
    w_lin16 = np.ascontiguousarray(W_lin, dtype=np.float32).astype(bf16)
    w_tr16 = np.ascontiguousarray(W_tr, dtype=np.float32).astype(bf16)
    btr_t = np.tile(np.asarray(b_tr, dtype=np.float32), T2)[None, :].astype(bf16)

    mergeT_full = np.zeros((P, PAD_N), dtype=bf16)
    mergeT_full[:, :N_NODES] = np.ascontiguousarray(
        np.asarray(merge, dtype=np.float32).T
    ).astype(bf16)

    structure = np.asarray(structure, dtype=np.int64)

    # phase-1 writes the h table in a partition-major permuted layout:
    # node n = st*T1*P + t*P + p lands at DRAM row st*T1*P + p*T1 + t.
    # Remap all gather indices to that layout.
    n_all = np.arange(PAD_N, dtype=np.int64)
    st_, q_ = np.divmod(n_all, T1 * P)
    t_, p_ = np.divmod(q_, P)
    perm = (st_ * T1 * P + p_ * T1 + t_).astype(np.int32)

    in_maps = []
    for k in range(N_CORES):
        lo = k * SHARD
        hi = min(lo + SHARD, N_NODES)
        idx = np.zeros((SHARD, DEG), dtype=np.int32)
        idx[: hi - lo] = perm[structure[lo:hi]]
        in_maps.append(
            {
                "dataT": dataT,
                "dataT_own": np.ascontiguousarray(dataT[:, lo : lo + SHARD]),
                "mergeT": np.ascontiguousarray(mergeT_full[:, lo : lo + SHARD]),
                "idx_r": idx,
                "w_lin": w_lin16,
                "w_tr": w_tr16,
                "btr_t": btr_t,
            }
        )
    return in_maps


_NC_CACHE = {}


def _get_nc():
    if "nc" not in _NC_CACHE:
        _NC_CACHE["nc"] = build_nc()
    return _NC_CACHE["nc"]


def kernel(data, merge, structure, W_lin, b_lin, W_tr, b_tr):
    from concourse.bass_utils import run_bass_kernel_spmd

    del b_lin  # cancels exactly in the normalized laplacian
    nc = _get_nc()
    in_maps = _make_in_maps(
        np.asarray(data),
        np.asarray(merge),
        np.asarray(structure),
        np.asarray(W_lin),
        np.asarray(W_tr),
        np.asarray(b_tr),
    )
    res = run_bass_kernel_spmd(nc, in_maps, core_ids=list(range(N_CORES)))
    global LAST_RESULTS
    LAST_RESULTS = res
    shards = []
    for k in range(N_CORES):
        o = np.asarray(res.results[k]["out_r"]).astype(np.float32)
        # un-permute: DRAM row su*T2*P + p*T2 + t holds node su*T2*P + t*P + p
        o = (
            o.reshape(N_ST2, P, T2, D_OUT)
            .transpose(0, 2, 1, 3)
            .reshape(SHARD, D_OUT)
        )
        shards.append(o)
    out = np.concatenate(shards, axis=0)
    return out[:N_NODES]


# revision 18
# speedup vs baseline: 1.0334x; 1.0334x over previous
"""Trainium2 Bass kernel for nn_ConvSkip (GNN message passing layer).

Computes, for the full graph:
    h    = data @ W_lin                 (b_lin cancels in the laplacian)
    lap  = h - (1/deg) * sum_{j in N(i)} h_j
    out  = relu(lap + merge @ W_tr + b_tr)

Sharding: nodes are padded to 50176 and split into 8 contiguous shards of
6272 (49 node-tiles of 128). Every core receives the SAME pre-transposed
bf16 feature table dataT [128, 50176] and redundantly computes the full
transformed table h (bf16) into its own DRAM, written in a host-permuted
row order that makes every table write a contiguous full-bandwidth DMA.
Each core then processes its own shard: per supertile it issues one
indirect-DMA gather per (node-tile, neighbor-slot) — the HW consumes one
offset per partition — reduces the 16 neighbor rows, and accumulates
bias + merge @ W_tr + data_own @ W_lin (the laplacian self term) in PSUM.

Per-core differences live entirely in the inputs (idx / mergeT /
dataT_own / out slices), so the compiled program is identical on all
cores and no node rotation is needed.
"""

import numpy as np

P = 128
N_NODES = 50000
DEG = 16
D_IN = 128
D_OUT = 64
N_CORES = 8
PAD_N = 50176  # 392 tiles of 128
SHARD = PAD_N // N_CORES  # 6272 = 49 tiles

# phase-1 supertile: T1 node-tiles of 128 rows per step (4 PSUM groups of 7)
T1 = 28
N_ST1 = PAD_N // (T1 * P)  # 14
# phase-2 supertile: T2 node-tiles of 128 rows per step
T2 = 7
N_ST2 = SHARD // (T2 * P)  # 7


def build_nc(repeat=1, bounds=None):
    """bounds: optional [N_ST2*T2*DEG] array of exact per-gather-slot upper
    row bounds (host-computed from the actual indices). A gather whose
    source AP covers only a prefix of the table can start before phase 1
    finishes writing the rest, overlapping gather desc-gen with the tail
    of the table build."""
    import concourse.bass as bass
    import concourse.tile as tile
    from concourse import bacc, mybir

    if bounds is None:
        bounds = [PAD_N] * (N_ST2 * T2 * DEG)

    f32 = mybir.dt.float32
    bf16 = mybir.dt.bfloat16
    i32 = mybir.dt.int32

    nc = bacc.Bacc("TRN2", target_bir_lowering=False)

    dataT = nc.declare_dram_parameter("dataT", [P, PAD_N], bf16, isOutput=False)
    dataT_own = nc.declare_dram_parameter("dataT_own", [P, SHARD], bf16, isOutput=False)
    mergeT = nc.declare_dram_parameter("mergeT", [P, SHARD], bf16, isOutput=False)
    idx_r = nc.declare_dram_parameter("idx_r", [SHARD, DEG], i32, isOutput=False)
    w_lin = nc.declare_dram_parameter("w_lin", [D_IN, D_OUT], bf16, isOutput=False)
    w_tr = nc.declare_dram_parameter("w_tr", [D_IN, D_OUT], bf16, isOutput=False)
    btr_t = nc.declare_dram_parameter("btr_t", [1, T2 * D_OUT], bf16, isOutput=False)
    out_r = nc.declare_dram_parameter("out_r", [SHARD, D_OUT], bf16, isOutput=True)

    with tile.TileContext(nc) as tc:
        with (
            tc.tile_pool(name="const", bufs=1) as cpool,
            tc.tile_pool(name="sbuf", bufs=2) as pool,
            tc.tile_pool(name="gath", bufs=3) as gpool,
            tc.tile_pool(name="ld", bufs=3) as ldpool,
            tc.tile_pool(name="idxp", bufs=7) as idxpool,
            tc.tile_pool(name="mp", bufs=4) as mpool,
            tc.tile_pool(name="psum", bufs=2, space="PSUM") as psum,
            tc.tile_pool(name="psum2", bufs=4, space="PSUM") as psum2,
            tc.tile_pool(name="dram", bufs=1, space="DRAM") as dpool,
        ):
            # ---- constants ----
            w_lin_sb = cpool.tile([P, D_OUT], bf16)
            nc.sync.dma_start(out=w_lin_sb[:], in_=w_lin[:, :])
            w_tr_sb = cpool.tile([P, D_OUT], bf16)
            nc.sync.dma_start(out=w_tr_sb[:], in_=w_tr[:, :])
            ones1 = cpool.tile([1, P], bf16)
            nc.vector.memset(ones1[:], 1.0)
            btr_sb = cpool.tile([1, T2 * D_OUT], bf16)
            nc.sync.dma_start(out=btr_sb[:], in_=btr_t[:, :])

            # full transformed-feature table in this core's DRAM (bf16)
            h_table = dpool.tile([PAD_N, D_OUT], bf16)

            def body():
                # preload all gather-offset tiles ahead of the phase-1 data
                # loads: they are tiny, and the bounded early gathers below
                # must not queue behind 12.8MB of feature loads on SP
                idx_sbs = []
                for su in range(N_ST2):
                    r0 = su * T2 * P
                    idx_sb = idxpool.tile([P, T2, DEG], i32, tag="idx")
                    nc.sync.dma_start(
                        out=idx_sb[:],
                        in_=idx_r[r0 : r0 + T2 * P, :].rearrange(
                            "(t p) d -> p t d", p=P
                        ),
                    )
                    idx_sbs.append(idx_sb)

                # ---- phase 1: h table over all nodes ----
                for st in range(N_ST1):
                    r0 = st * T1 * P
                    xT = ldpool.tile([P, T1 * P], bf16, tag="ld1")
                    nc.sync.dma_start(out=xT[:], in_=dataT[:, r0 : r0 + T1 * P])
                    stage = pool.tile([P, T1, D_OUT], bf16, tag="stage")
                    for g in range(4):
                        h_ps = psum.tile([P, T2 * D_OUT], f32, tag="mm")
                        for t in range(T2):
                            j = g * T2 + t
                            nc.tensor.matmul(
                                out=h_ps[:, t * D_OUT : (t + 1) * D_OUT],
                                lhsT=xT[:, j * P : (j + 1) * P],
                                rhs=w_lin_sb[:],
                                start=True,
                                stop=True,
                            )
                        if (st * 4 + g) % 2 == 0:
                            nc.vector.tensor_copy(
                                out=stage[:, g * T2 : (g + 1) * T2, :],
                                in_=h_ps[:].rearrange("p (t f) -> p t f", t=T2),
                            )
                        else:
                            nc.scalar.copy(
                                out=stage[:, g * T2 : (g + 1) * T2, :],
                                in_=h_ps[:].rearrange("p (t f) -> p t f", t=T2),
                            )
                    # permuted table layout: node n = r0 + t*128 + p is stored
                    # at DRAM row r0 + p*T1 + t, making each partition's write
                    # a single contiguous 3584B chunk (full DMA bandwidth).
                    # The host permutes gather indices to match.
                    nc.scalar.dma_start(
                        out=h_table[r0 : r0 + T1 * P, :].rearrange(
                            "(p t) f -> p t f", p=P
                        ),
                        in_=stage[:],
                    )

                # ---- phase 2: gather + laplacian + skip for own shard ----
                for su in range(N_ST2):
                    r0 = su * T2 * P
                    idx_sb = idx_sbs[su]
                    m_sb = mpool.tile([P, T2 * P], bf16, tag="ldm")
                    nc.sync.dma_start(out=m_sb[:], in_=mergeT[:, r0 : r0 + T2 * P])
                    o_sb = mpool.tile([P, T2 * P], bf16, tag="ldo")
                    nc.sync.dma_start(out=o_sb[:], in_=dataT_own[:, r0 : r0 + T2 * P])

                    # gather: HW indirect DMA consumes ONE offset per partition
                    # (streams contiguously per partition), so issue one gather
                    # per (node-tile, neighbor-slot): [128,1] idx -> [128,64].
                    # issue d-major: slots are host-sorted by table row, so
                    # low d has small bounds everywhere — on the in-order Pool
                    # queue this lets early slots start while phase 1 still
                    # writes the tail of the table.
                    gath = gpool.tile([P, T2 * DEG * D_OUT], bf16, tag="gath")
                    for dd in range(DEG):
                        for t in range(T2):
                            k = t * DEG + dd
                            bnd = bounds[(su * T2 + t) * DEG + dd]
                            nc.gpsimd.indirect_dma_start(
                                out=gath[:, k * D_OUT : (k + 1) * D_OUT],
                                out_offset=None,
                                in_=h_table[0:bnd, :],
                                in_offset=bass.IndirectOffsetOnAxis(
                                    ap=idx_sb[:, t, dd : dd + 1], axis=0
                                ),
                            )

                    # skip branch + laplacian self term, accumulated in PSUM:
                    # sk = b_tr + merge @ W_tr + data_own @ W_lin
                    sk_ps = psum2.tile([P, T2 * D_OUT], f32, tag="sk")
                    nc.tensor.matmul(
                        out=sk_ps[:],
                        lhsT=ones1[:],
                        rhs=btr_sb[:],
                        start=True,
                        stop=False,
                        skip_group_check=True,
                    )
                    for t in range(T2):
                        nc.tensor.matmul(
                            out=sk_ps[:, t * D_OUT : (t + 1) * D_OUT],
                            lhsT=m_sb[:, t * P : (t + 1) * P],
                            rhs=w_tr_sb[:],
                            start=False,
                            stop=False,
                            skip_group_check=True,
                        )
                        nc.tensor.matmul(
                            out=sk_ps[:, t * D_OUT : (t + 1) * D_OUT],
                            lhsT=o_sb[:, t * P : (t + 1) * P],
                            rhs=w_lin_sb[:],
                            start=False,
                            stop=(t == T2 - 1),
                            skip_group_check=True,
                        )

                    nsum = pool.tile([P, T2, D_OUT], f32, tag="nsum")
                    nc.vector.reduce_sum(
                        out=nsum[:],
                        in_=gath[:].rearrange("p (t d f) -> p t f d", t=T2, d=DEG),
                        axis=mybir.AxisListType.X,
                    )
                    osum = pool.tile([P, T2, D_OUT], f32, tag="osum")
                    nc.vector.scalar_tensor_tensor(
                        out=osum[:],
                        in0=nsum[:],
                        scalar=-1.0 / DEG,
                        in1=sk_ps[:].rearrange("p (t f) -> p t f", t=T2),
                        op0=mybir.AluOpType.mult,
                        op1=mybir.AluOpType.add,
                    )
                    orelu = pool.tile([P, T2, D_OUT], bf16, tag="orelu")
                    nc.scalar.activation(
                        out=orelu[:],
                        in_=osum[:],
                        func=mybir.ActivationFunctionType.Relu,
                    )
                    # permuted output rows (node su*896 + t*128 + p -> row
                    # su*896 + p*T2 + t); host un-permutes. 896B contiguous
                    # per partition avoids the small-descriptor DMA penalty.
                    nc.scalar.dma_start(
                        out=out_r[r0 : r0 + T2 * P, :].rearrange("(p t) f -> p t f", p=P),
                        in_=orelu[:],
                    )

            for _rep in range(repeat):
                body()

    nc.finalize()
    return nc


def _make_in_maps(data, merge, structure, W_lin, W_tr, b_tr):
    import ml_dtypes

    bf16 = ml_dtypes.bfloat16

    data_f = np.ascontiguousarray(data, dtype=np.float32)
    dataT = np.zeros((P, PAD_N), dtype=bf16)
    dataT[:, :N_NODES] = np.ascontiguousarray(data_f.T).astype(bf16)
    w_lin16 = np.ascontiguousarray(W_lin, dtype=np.float32).astype(bf16)
    w_tr16 = np.ascontiguousarray(W_tr, dtype=np.float32).astype(bf16)
    btr_t = np.tile(np.asarray(b_tr, dtype=np.float32), T2)[None, :].astype(bf16)

    mergeT_full = np.zeros((P, PAD_N), dtype=bf16)
    mergeT_full[:, :N_NODES] = np.ascontiguousarray(
        np.asarray(merge, dtype=np.float32).T
    ).astype(bf16)

    structure = np.asarray(structure, dtype=np.int64)

    # phase-1 writes the h table in a partition-major permuted layout:
    # node n = st*T1*P + t*P + p lands at DRAM row st*T1*P + p*T1 + t.
    # Remap all gather indices to that layout.
    n_all = np.arange(PAD_N, dtype=np.int64)
    st_, q_ = np.divmod(n_all, T1 * P)
    t_, p_ = np.divmod(q_, P)
    perm = (st_ * T1 * P + p_ * T1 + t_).astype(np.int32)

    in_maps = []
    all_idx = []
    for k in range(N_CORES):
        lo = k * SHARD
        hi = min(lo + SHARD, N_NODES)
        idx = np.zeros((SHARD, DEG), dtype=np.int32)
        # sort each node's slots by permuted table row: the sum over
        # neighbors is order-free, and sorted slots give small exact
        # per-slot table bounds so early gathers overlap the table build
        idx[: hi - lo] = np.sort(perm[structure[lo:hi]], axis=1)
        all_idx.append(idx)
        in_maps.append(
            {
                "dataT": dataT,
                "dataT_own": np.ascontiguousarray(dataT[:, lo : lo + SHARD]),
                "mergeT": np.ascontiguousarray(mergeT_full[:, lo : lo + SHARD]),
                "idx_r": idx,
                "w_lin": w_lin16,
                "w_tr": w_tr16,
                "btr_t": btr_t,
            }
        )
    # exact per-(supertile, tile, slot) table bound, max over all cores
    # (the SPMD program is shared), rounded up to a phase-1 write boundary
    stk = np.stack(all_idx)  # [cores, SHARD, DEG]
    stk = stk.reshape(N_CORES, N_ST2, T2, P, DEG)
    bmax = stk.max(axis=(0, 3)) + 1  # [N_ST2, T2, DEG]
    step = T1 * P
    bounds = tuple(
        int(min(PAD_N, -(-b // step) * step)) for b in bmax.reshape(-1)
    )
    return in_maps, bounds


_NC_CACHE = {}


def _get_nc(bounds=None):
    key = bounds
    if key not in _NC_CACHE:
        _NC_CACHE[key] = build_nc(bounds=bounds)
    return _NC_CACHE[key]


def kernel(data, merge, structure, W_lin, b_lin, W_tr, b_tr):
    from concourse.bass_utils import run_bass_kernel_spmd

    del b_lin  # cancels exactly in the normalized laplacian
    in_maps, bounds = _make_in_maps(
        np.asarray(data),
        np.asarray(merge),
        np.asarray(structure),
        np.asarray(W_lin),
        np.asarray(W_tr),
        np.asarray(b_tr),
    )
    nc = _get_nc(bounds)
    res = run_bass_kernel_spmd(nc, in_maps, core_ids=list(range(N_CORES)))
    global LAST_RESULTS
    LAST_RESULTS = res
    shards = []
    for k in range(N_CORES):
        o = np.asarray(res.results[k]["out_r"]).astype(np.float32)
        # un-permute: DRAM row su*T2*P + p*T2 + t holds node su*T2*P + t*P + p
        o = (
            o.reshape(N_ST2, P, T2, D_OUT)
            .transpose(0, 2, 1, 3)
            .reshape(SHARD, D_OUT)
        )
        shards.append(o)
    out = np.concatenate(shards, axis=0)
    return out[:N_NODES]
